# revision 18
# baseline (speedup 1.0000x reference)
"""GPT-style transformer forward on 8 Trainium2 NeuronCores.

Sharding: data-parallel over batch (2 groups of 4 cores), tensor-parallel
within each group (heads / FFN hidden / vocab columns split 4 ways).
Device activations are feature-major [feature, token] so all matmuls run
without transposes.

v2: everything is processed in 2 token blocks of 512 and software-pipelined
so the per-block AllReduces overlap the other block's compute.  Row
broadcasts run on GpSimd (partition_broadcast) instead of K=1 matmuls,
reciprocals use the fast-approx DVE op, causal masking is a GpSimd
affine_select directly on the exp() output, the residual accumulator is
bf16 so LN stats matmuls read it without a copy, and the vocab phase keeps
logits in SBUF with per-block log-softmax pipelining (bf16 output, host
casts to f32).
"""

import os
from contextlib import ExitStack

import numpy as np
import ml_dtypes

import concourse.bass as bass
import concourse.bass_utils as _bu

# walrus disables the LDWEIGHTS pull-ahead optimization by default; without
# it every matmul serializes behind its weight load (~+100ns/MM).  Re-enable
# it for this kernel's compiles (gated by env for A/B testing).
if os.environ.get("BASS_GPT_LDWOPT", "0") == "1":
    _orig_run_command = _bu.run_command

    def _run_command_ldw(cmd, **kw):
        cmd = [
            c.replace("--enable-ldw-opt=false", "--enable-ldw-opt=true")
            if isinstance(c, str) else c
            for c in cmd
        ]
        return _orig_run_command(cmd, **kw)

    _bu.run_command = _run_command_ldw
import concourse.mybir as mybir
import concourse.tile as tile
from concourse.bass_utils import run_bass_kernel_spmd
from concourse.vector_clock import ScopedClock


def _drain_and_barrier(self, tick_clock, wait_clock):
    """The walrus build here encodes Drain/NoOp as TPB_CTRL with at most one
    sync-wait slot; Tile's stock tail attaches all outstanding waits to the
    Drain and fails codegen. Split the waits one-per-NOP instead."""
    nop_inst = self.nc.sync.nop(nofuse=True)
    wait_clock.add_sem_waits(nop_inst.ins, ScopedClock({None: tick_clock.global_clock}))
    si = nop_inst.ins.sync_info
    if si is not None and len(si.on_wait) > 1:
        waits = list(si.on_wait)
        nop_inst.ins.sync_info = mybir.SyncInfo(on_wait=waits[:1], on_update=list(si.on_update))
        for w in waits[1:]:
            n2 = self.nc.sync.nop(nofuse=True)
            n2.ins.sync_info = mybir.SyncInfo(on_wait=[w], on_update=[])
    self.nc.sync.drain()
    self.nc.all_engine_barrier()
    assert self.sems is not None
    popped = self.nc._tile_sem_poison_stack.pop()
    assert popped is self._sem_poison
    self.nc.clear_and_free_semaphores(list(self.sems.allocated().values()))
    self.nc.all_engine_barrier()


tile.TileContext._drain_and_barrier = _drain_and_barrier

_MAX_WAITS = 1  # this walrus build caps sync-waits per instruction


def split_sync_waits(nc):
    """Hoist excess on_wait entries onto same-engine NOPs inserted before the
    instruction (engine queues execute in program order, so semantics hold)."""
    n = 0
    for bb in nc.main_func.blocks:
        insts = bb.instructions
        new_list = []
        for inst in insts:
            si = getattr(inst, "sync_info", None)
            # the LDW-opt codegen pass rejects Ldweights carrying sync waits;
            # hoist ALL of them (same-engine NOP preserves ordering).
            keep = 0 if isinstance(inst, mybir.InstLdweights) else _MAX_WAITS
            if si is not None and len(si.on_wait) > keep:
                waits = list(si.on_wait)
                cut = len(waits) - keep
                nops = []
                for w in waits[:cut]:
                    n += 1
                    nops.append(mybir.InstNoOp(
                        name=f"{inst.name}-sw{n}",
                        sync_info=mybir.SyncInfo(on_wait=[w], on_update=[]),
                        bass_nofuse=True,
                        engine=inst.engine,
                    ))
                # never split an Ldweights from its matmul: hoist the NOPs
                # above any trailing Ldweights run (waits only become more
                # conservative when moved earlier in an in-order queue)
                ip = len(new_list)
                while ip > 0 and isinstance(new_list[ip - 1], mybir.InstLdweights):
                    ip -= 1
                new_list[ip:ip] = nops
                if keep == 0 and not si.on_update:
                    inst.sync_info = None
                else:
                    inst.sync_info = mybir.SyncInfo(
                        on_wait=waits[cut:], on_update=list(si.on_update)
                    )
            new_list.append(inst)
        if len(new_list) != len(insts):
            bb.instructions[:] = new_list
    return n


# Model dims (hardcoded per problem spec)
L_FULL, H, D, V, SMAX = 8, 16, 1024, 32000, 1024
DH = D // H          # 64
FF = 4 * D           # 4096
B, S = 2, 1024
T = S                # tokens per group (one batch element per group)
TP = 4               # tensor-parallel degree within a group
HL = H // TP         # 4 local heads
FFL = FF // TP       # 1024 local FFN cols
VL = V // TP         # 8000 local vocab cols
VLP = 8064           # padded to 63*128
NVM = VLP // 128     # 63 vocab m-tiles
EPS = 1e-5
KT = D // 128        # 8 k-tiles over model dim
NB = T // 512        # 2 token blocks of 512

BF = mybir.dt.bfloat16
F8 = mybir.dt.float8e4
DRM = mybir.MatmulPerfMode.DoubleRow
KP = 4            # k-pair tiles (KT//2) for DoubleRow
WS = 1024.0       # fp8 weight scale
AS = 16.0         # fp8 activation scale
DS = 1.0 / (WS * AS)
F32 = mybir.dt.float32
AF = mybir.ActivationFunctionType
ALU = mybir.AluOpType

RG = [[0, 1, 2, 3], [4, 5, 6, 7]]

N_LAYERS = int(os.environ.get("BASS_GPT_LAYERS", str(L_FULL)))
SKIP_FINAL = os.environ.get("BASS_GPT_SKIP_FINAL", "0") == "1"

# packed per-layer bias/scale column offsets in bm_sb [128, 62]
BQKV, BO, G1, BB1, B1C, B2C, G2, BB2 = 0, 6, 14, 22, 30, 38, 46, 54
NBM = 62


def _r2(ap):
    """[ (kt p) n ] -> [p kt n] view of a DRAM 2-D tensor (p=128)."""
    return ap.rearrange("(kt p) n -> p kt n", p=128)


def build_program():
    nc = bass.Bass("TRN2")

    # ---- DRAM parameters (per-core shards) ----
    h0 = nc.declare_dram_parameter("h0", [D, T], BF, isOutput=False)
    wqkv = nc.declare_dram_parameter("wqkv", [N_LAYERS, D, 3 * HL * DH], BF, isOutput=False)
    wo = nc.declare_dram_parameter("wo", [N_LAYERS, HL * DH, D], BF, isOutput=False)
    w1 = nc.declare_dram_parameter("w1", [N_LAYERS, D, FFL], BF, isOutput=False)
    w2 = nc.declare_dram_parameter("w2", [N_LAYERS, FFL, D], BF, isOutput=False)
    bm = nc.declare_dram_parameter("bm", [N_LAYERS, 128, NBM], F32, isOutput=False)
    wout = nc.declare_dram_parameter("wout", [D, VLP], BF, isOutput=False)
    bout = nc.declare_dram_parameter("bout", [VLP], F32, isOutput=False)
    out = nc.declare_dram_parameter("out", [VLP, T], BF, isOutput=True)

    with ExitStack() as ctx:
        tc = ctx.enter_context(tile.TileContext(nc))

        const = ctx.enter_context(tc.tile_pool(name="const", bufs=1))
        hpool = ctx.enter_context(tc.tile_pool(name="hpool", bufs=1))
        rpool = ctx.enter_context(tc.tile_pool(name="rpool", bufs=4))
        dram = ctx.enter_context(tc.tile_pool(name="dram", bufs=2, space="DRAM"))

        # ---- constants ----
        ones_k = const.tile([128, 1], BF)       # lhsT for partition-sum (K=128, M=1)
        nc.vector.memset(ones_k, 1.0)
        eps_sb = const.tile([1, 1], F32)
        nc.vector.memset(eps_sb, float(D * D * EPS))
        ones_m = const.tile([1, 128], BF)       # lhsT for bf16 broadcast (K=1)
        nc.vector.memset(ones_m, 1.0)
        ones_mf = const.tile([1, 128], F32)     # lhsT for f32 broadcast
        nc.vector.memset(ones_mf, 1.0)
        lnD = const.tile([1, 1], F32)
        nc.vector.memset(lnD, float(np.log(D)))
        salt = os.environ.get("BASS_GPT_SALT", "")
        if salt:  # perturb the BIR so the NEFF cache misses on flag changes
            saltt = const.tile([1, 1], F32)
            nc.vector.memset(saltt, float(1 + len(salt)))

        # ---- persistent activation state ----
        hb = hpool.tile([128, KT, T], BF)       # residual stream (feature-major)
        nc.sync.dma_start(hb, _r2(h0))

        with ExitStack() as lctx:
            xpool = lctx.enter_context(tc.tile_pool(name="xpool", bufs=1))
            sqpool = lctx.enter_context(tc.tile_pool(name="sqpool", bufs=1))
            apool = lctx.enter_context(tc.tile_pool(name="apool", bufs=1))
            epool = lctx.enter_context(tc.tile_pool(name="epool", bufs=3))
            wq_pool = lctx.enter_context(tc.tile_pool(name="wq_pool", bufs=2))
            wf_pool = lctx.enter_context(tc.tile_pool(name="wf_pool", bufs=2))
            bpool = lctx.enter_context(tc.tile_pool(name="bpool", bufs=2))
            spool = lctx.enter_context(tc.tile_pool(name="spool", bufs=2))
            fpool = lctx.enter_context(tc.tile_pool(name="fpool", bufs=1))

            mm_psum = lctx.enter_context(tc.tile_pool(name="mm_psum", bufs=3, space="PSUM"))
            o_psum = lctx.enter_context(tc.tile_pool(name="o_psum", bufs=1, space="PSUM"))
            st_psum = lctx.enter_context(tc.tile_pool(name="st_psum", bufs=1, space="PSUM"))
            bc_psum = lctx.enter_context(tc.tile_pool(name="bc_psum", bufs=1, space="PSUM"))

            x1b = xpool.tile([128, KT, T], BF, tag="x1b")   # pre-LN accumulator
            qk_sb = apool.tile([128, 2, 2, T], BF)  # [part, q/k, head-pair, t]
            vaug = apool.tile([128, KT, HL, 65], BF)  # token-major V + ones col
            oT = apool.tile([128, 2, T], BF)        # attn head outputs (feature-major)
            f1 = fpool.tile([128, KT, T], BF)       # FFN hidden (local)

            def layernorm(blk, gcol, bcol):
                """LN over features of x1b block -> writes hb block (bf16)."""
                tsl = slice(blk * 512, (blk + 1) * 512)
                xs = sqpool.tile([128, KT, 512], BF, tag="xsq")
                nc.vector.tensor_mul(xs, x1b[:, :, tsl], x1b[:, :, tsl])
                s1p = st_psum.tile([1, 512], F32, tag="s1")
                s2p = st_psum.tile([33, 512], F32, tag="s2")
                for kt in range(KT):
                    nc.tensor.matmul(s1p, ones_k, x1b[:, kt, tsl],
                                     start=(kt == 0), stop=(kt == KT - 1))
                    nc.tensor.matmul(s2p[32:33, :], ones_k, xs[:, kt, :],
                                     start=(kt == 0), stop=(kt == KT - 1))
                # u = s2*D - s1^2 + D^2*eps ; rstd = D/sqrt(u) = exp(-ln(u)/2 + ln D)
                t0 = rpool.tile([1, 512], F32, tag="row")
                nc.scalar.activation(t0, s1p, AF.Square)
                t1 = rpool.tile([1, 512], F32, tag="row")
                nc.vector.scalar_tensor_tensor(
                    out=t1, in0=s2p[32:33, :], scalar=float(D), in1=t0,
                    op0=ALU.mult, op1=ALU.subtract,
                )
                lu = rpool.tile([1, 512], F32, tag="row")
                nc.scalar.activation(lu, t1, AF.Ln, bias=eps_sb[0:1, 0:1])
                a2 = rpool.tile([1, 512], BF, tag="row")
                nc.scalar.activation(a2, lu, AF.Exp, scale=-0.5, bias=lnD[0:1, 0:1])
                c2 = rpool.tile([1, 512], BF, tag="row")
                nc.vector.scalar_tensor_tensor(
                    out=c2, in0=s1p, scalar=float(-1.0 / D), in1=a2,
                    op0=ALU.mult, op1=ALU.mult,
                )
                bcA = bc_psum.tile([128, 512], F32, tag="bc")
                nc.tensor.matmul(bcA, ones_m, a2, start=True, stop=True)
                A = spool.tile([128, 512], BF, tag="Abc")
                nc.vector.tensor_copy(A, bcA)
                bcC = bc_psum.tile([128, 512], F32, tag="bc")
                nc.tensor.matmul(bcC, ones_m, c2, start=True, stop=True)
                C = spool.tile([128, 512], BF, tag="Cbc")
                nc.vector.tensor_copy(C, bcC)
                for kt in range(KT):
                    Cp = spool.tile([128, 512], BF, tag="Cp")
                    nc.vector.tensor_scalar(
                        out=Cp, in0=C, scalar1=gcol[:, kt : kt + 1],
                        scalar2=bcol[:, kt : kt + 1], op0=ALU.mult, op1=ALU.add,
                    )
                    u = spool.tile([128, 512], BF, tag="u")
                    nc.vector.scalar_tensor_tensor(
                        out=u, in0=x1b[:, kt, tsl], scalar=gcol[:, kt : kt + 1],
                        in1=A, op0=ALU.mult, op1=ALU.mult,
                    )
                    nc.vector.tensor_add(hb[:, kt, tsl], u, Cp)

            def load_attn_w(l):
                wqkv_sb = wq_pool.tile([128, KT, 768], BF, tag="wqkv")
                nc.sync.dma_start(wqkv_sb, _r2(wqkv[l]))
                wo_sb = wq_pool.tile([128, 2, D], BF, tag="wo")
                nc.sync.dma_start(wo_sb, _r2(wo[l]))
                bm_sb = bpool.tile([128, NBM], F32, tag="bm")
                nc.sync.dma_start(bm_sb, bm[l])
                return wqkv_sb, wo_sb, bm_sb

            def load_ffn_w(l):
                w1_sb = wf_pool.tile([128, KT, FFL], BF, tag="wf")
                nc.sync.dma_start(w1_sb, _r2(w1[l]))
                w2_sb = wf_pool.tile([128, KT, D], BF, tag="wf")
                nc.sync.dma_start(w2_sb, _r2(w2[l]))
                return w1_sb, w2_sb

            def qkv_attn_out(blk, W):
                """QKV + attention + out-proj + AllReduce for one token block."""
                wqkv_sb, wo_sb, bm_sb = W
                tsl = slice(blk * 512, (blk + 1) * 512)
                nc.gpsimd.memset(vaug[:, 4 * blk : 4 * blk + 4, :, 64:65], 1.0)
                for io in range(2):        # 0=q, 1=k (feature-major out)
                    for mt in range(2):    # head pair
                        mcol = (io * 2 + mt) * 128
                        ps = mm_psum.tile([128, 512], F32, tag="mm")
                        for kt in range(KT):
                            nc.tensor.matmul(
                                ps, wqkv_sb[:, kt, mcol : mcol + 128],
                                hb[:, kt, tsl],
                                start=(kt == 0), stop=(kt == KT - 1),
                            )
                        nc.scalar.activation(
                            qk_sb[:, io, mt, tsl], ps, AF.Identity,
                            bias=bm_sb[:, BQKV + io * 2 + mt : BQKV + io * 2 + mt + 1],
                        )
                for tm in range(4 * blk, 4 * blk + 4):  # v, token-major
                    ps = mm_psum.tile([128, 256], F32, tag="mm")
                    for kt in range(KT):
                        nc.tensor.matmul(
                            ps, hb[:, kt, tm * 128 : (tm + 1) * 128],
                            wqkv_sb[:, kt, 512:768],
                            start=(kt == 0), stop=(kt == KT - 1),
                        )
                    nc.scalar.activation(
                        vaug[:, tm, :, 0:64],
                        ps.rearrange("p (h e) -> p h e", h=HL), AF.Copy,
                    )
                # attention
                t1sl = tsl
                t2max = 4 * (blk + 1)
                for hm in range(2):
                    ps_oA = o_psum.tile([65, 512], F32, tag="oA")
                    ps_oB = o_psum.tile([65, 512], F32, tag="oB")
                    ps_o = [ps_oA, ps_oB]
                    ets = {}
                    for t2t in range(t2max + 1):
                        if t2t < t2max:
                            et = epool.tile([128, 2, 512], BF, tag="et")
                            ets[t2t] = et
                            for p in range(2):
                                pr = slice(64 * p, 64 * p + 64)
                                ps = mm_psum.tile([128, 512], F32, tag="mm")
                                nc.tensor.matmul(
                                    ps,
                                    qk_sb[pr, 1, hm, t2t * 128 : (t2t + 1) * 128],
                                    qk_sb[pr, 0, hm, t1sl],
                                    start=True, stop=True,
                                )
                                nc.scalar.activation(et[:, p, :], ps, AF.Exp, scale=0.125)
                            j = t2t - 4 * blk
                            if j >= 0:
                                nc.gpsimd.affine_select(
                                    out=et, in_=et,
                                    compare_op=ALU.is_ge, fill=0.0,
                                    base=-128 * j, pattern=[[0, 2], [1, 512]],
                                    channel_multiplier=-1,
                                )
                        if t2t >= 1:  # AV lags one tile so it never waits on exp
                            ta = t2t - 1
                            eta = ets.pop(ta)
                            for p in range(2):
                                nc.tensor.matmul(
                                    ps_o[p], vaug[:, ta, 2 * hm + p, :],
                                    eta[:, p, :],
                                    start=(ta == 0), stop=(ta == t2max - 1),
                                )
                    for p in range(2):
                        pr = slice(64 * p, 64 * p + 64)
                        lden = rpool.tile([1, 512], F32, tag="rec")
                        nc.scalar.activation(lden, ps_o[p][64:65, :], AF.Ln)
                        rcb = rpool.tile([1, 512], BF, tag="rec")
                        nc.scalar.activation(rcb, lden, AF.Exp, scale=-1.0)
                        bcR = bc_psum.tile([64, 512], F32, tag="bc")
                        nc.tensor.matmul(bcR, ones_m[:, 0:64], rcb, start=True, stop=True)
                        rb = rpool.tile([64, 512], BF, tag="rb")
                        nc.vector.tensor_copy(rb, bcR)
                        tmp = spool.tile([64, 512], BF, tag="otmp")
                        nc.vector.tensor_mul(tmp, ps_o[p][0:64, :], rb)
                        nc.vector.tensor_scalar(
                            out=oT[pr, hm, t1sl], in0=tmp,
                            scalar1=bm_sb[pr, BQKV + 4 + hm : BQKV + 5 + hm],
                            scalar2=None, op0=ALU.add,
                        )
                # out-proj + AllReduce
                stage = spool.tile([128, KT, 512], BF, tag="stage")
                for mt in range(KT):
                    ps = mm_psum.tile([128, 512], F32, tag="mm")
                    for kt in range(2):
                        nc.tensor.matmul(
                            ps, wo_sb[:, kt, mt * 128 : (mt + 1) * 128],
                            oT[:, kt, tsl],
                            start=(kt == 0), stop=(kt == 1),
                        )
                    nc.scalar.activation(
                        stage[:, mt, :], ps, AF.Identity,
                        bias=bm_sb[:, BO + mt : BO + mt + 1],
                    )
                ar_in = dram.tile([D, 512], BF, tag="arin")
                nc.sync.dma_start(_r2(ar_in), stage)
                ar_out = dram.tile([D, 512], BF, tag="arout")
                nc.gpsimd.collective_compute(
                    "AllReduce", ALU.add, replica_groups=RG,
                    ins=[ar_in.opt()], outs=[ar_out.opt()],
                )
                return ar_out

            def residual_in(blk, ar_out):
                tsl = slice(blk * 512, (blk + 1) * 512)
                ar_sb = spool.tile([128, KT, 512], BF, tag="stage")
                nc.sync.dma_start(ar_sb, _r2(ar_out))
                nc.vector.tensor_add(x1b[:, :, tsl], ar_sb, hb[:, :, tsl])

            def ln_ffn(blk, W, Wf, ar):
                """residual + LN1 + FFN + AllReduce for one token block."""
                bm_sb = W[2]
                w1_sb, w2_sb = Wf
                residual_in(blk, ar)
                layernorm(blk, bm_sb[:, G1:G1 + KT], bm_sb[:, BB1:BB1 + KT])
                tsl = slice(blk * 512, (blk + 1) * 512)
                for mt in range(KT):
                    ps = mm_psum.tile([128, 512], F32, tag="mm")
                    for kt in range(KT):
                        nc.tensor.matmul(
                            ps, w1_sb[:, kt, mt * 128 : (mt + 1) * 128],
                            hb[:, kt, tsl],
                            start=(kt == 0), stop=(kt == KT - 1),
                        )
                    nc.vector.tensor_scalar(
                        out=f1[:, mt, tsl], in0=ps,
                        scalar1=bm_sb[:, B1C + mt : B1C + mt + 1], scalar2=0.0,
                        op0=ALU.add, op1=ALU.max,
                    )
                stage = spool.tile([128, KT, 512], BF, tag="stage")
                for mt in range(KT):
                    ps = mm_psum.tile([128, 512], F32, tag="mm")
                    for kt in range(KT):
                        nc.tensor.matmul(
                            ps, w2_sb[:, kt, mt * 128 : (mt + 1) * 128],
                            f1[:, kt, tsl],
                            start=(kt == 0), stop=(kt == KT - 1),
                        )
                    nc.vector.tensor_scalar(
                        out=stage[:, mt, :], in0=ps,
                        scalar1=bm_sb[:, B2C + mt : B2C + mt + 1],
                        scalar2=None, op0=ALU.add,
                    )
                ar_in = dram.tile([D, 512], BF, tag="arin")
                nc.sync.dma_start(_r2(ar_in), stage)
                ar_out = dram.tile([D, 512], BF, tag="arout")
                nc.gpsimd.collective_compute(
                    "AllReduce", ALU.add, replica_groups=RG,
                    ins=[ar_in.opt()], outs=[ar_out.opt()],
                )
                return ar_out

            def ln2_block(blk, W, ar):
                bm_sb = W[2]
                residual_in(blk, ar)
                layernorm(blk, bm_sb[:, G2:G2 + KT], bm_sb[:, BB2:BB2 + KT])

            # software pipeline across blocks AND the layer boundary so the
            # Tensor stream always has AR-independent work queued behind
            # each collective (engine queues are static FIFOs).
            W = load_attn_w(0)
            arA0 = qkv_attn_out(0, W)
            for l in range(N_LAYERS):
                arA1 = qkv_attn_out(1, W)
                Wf = load_ffn_w(l)
                arF0 = ln_ffn(0, W, Wf, arA0)
                arF1 = ln_ffn(1, W, Wf, arA1)
                ln2_block(0, W, arF0)
                if l + 1 < N_LAYERS:
                    Wn = load_attn_w(l + 1)
                    arA0 = qkv_attn_out(0, Wn)
                    ln2_block(1, W, arF1)
                    W = Wn
                else:
                    ln2_block(1, W, arF1)

        # ---- Phase G: vocab projection + log-softmax (layer pools closed) ----
        if not SKIP_FINAL:
            with ExitStack() as gctx:
                gpool = gctx.enter_context(tc.tile_pool(name="gpool", bufs=1))
                wch_pool = gctx.enter_context(tc.tile_pool(name="wch", bufs=3))
                ebpool = gctx.enter_context(tc.tile_pool(name="eb", bufs=3))
                ospool = gctx.enter_context(tc.tile_pool(name="osp", bufs=4))
                gmm_psum = gctx.enter_context(tc.tile_pool(name="gmm", bufs=5, space="PSUM"))
                gst_psum = gctx.enter_context(tc.tile_pool(name="gst", bufs=1, space="PSUM"))
                gbc_psum = gctx.enter_context(tc.tile_pool(name="gbc", bufs=1, space="PSUM"))

                bout_sb = gpool.tile([128, NVM], F32)
                nc.sync.dma_start(bout_sb, bout.rearrange("(m p) -> p m", p=128))
                logits = gpool.tile([128, NVM, NB, 512], BF)
                for nb in range(NB):
                    tsl = slice(nb * 512, (nb + 1) * 512)
                    if nb == 0:
                        acc = gst_psum.tile([1, 512], F32, tag="acc0")
                        acc_ap = acc
                    else:
                        acc = gst_psum.tile([33, 512], F32, tag="acc1")
                        acc_ap = acc[32:33, :]
                    for vm in range(NVM):
                        wv_sb = wch_pool.tile([128, KT, 128], BF, tag="wch")
                        nc.sync.dma_start(wv_sb, _r2(wout)[:, :, vm * 128 : (vm + 1) * 128])
                        ps = gmm_psum.tile([128, 512], F32, tag="mm")
                        for kt in range(KT):
                            nc.tensor.matmul(
                                ps, wv_sb[:, kt, :], hb[:, kt, tsl],
                                start=(kt == 0), stop=(kt == KT - 1),
                            )
                        nc.vector.tensor_scalar(
                            out=logits[:, vm, nb, :], in0=ps,
                            scalar1=bout_sb[:, vm : vm + 1], scalar2=None, op0=ALU.add,
                        )
                        eb = ebpool.tile([128, 512], BF, tag="eb")
                        nc.scalar.activation(eb, ps, AF.Exp, bias=bout_sb[:, vm : vm + 1])
                        nc.tensor.matmul(
                            acc_ap, ones_k, eb,
                            start=(vm == 0), stop=(vm == NVM - 1), skip_group_check=True,
                        )
                    se_row = rpool.tile([1, 512], F32, tag="row")
                    nc.vector.tensor_copy(se_row, acc_ap)
                    se_in = dram.tile([1, 512], F32, tag="sein")
                    nc.sync.dma_start(se_in, se_row)
                    se_out = dram.tile([1, 512], F32, tag="seout")
                    nc.gpsimd.collective_compute(
                        "AllReduce", ALU.add, replica_groups=RG,
                        ins=[se_in.opt()], outs=[se_out.opt()],
                    )
                    se_sb = rpool.tile([1, 512], F32, tag="row")
                    nc.sync.dma_start(se_sb, se_out)
                    lr = rpool.tile([1, 512], BF, tag="row")
                    nc.scalar.activation(lr, se_sb, AF.Ln)
                    Lb = gbc_psum.tile([128, 512], F32, tag="gbc")
                    nc.tensor.matmul(Lb, ones_m, lr, start=True, stop=True)
                    Lsb = gpool.tile([128, 512], BF, tag=f"Lsb{nb}")
                    nc.vector.tensor_copy(Lsb, Lb)
                    for vm in range(NVM):
                        of = ospool.tile([128, 512], BF, tag="of")
                        nc.vector.tensor_sub(of, logits[:, vm, nb, :], Lsb)
                        nc.sync.dma_start(out[vm * 128 : (vm + 1) * 128, tsl], of)
        else:
            # debug: dump hb into out rows
            for kt in range(KT):
                dbg = spool.tile([128, T], BF, tag="stage")
                nc.scalar.activation(dbg, hb[:, kt, :], AF.Copy)
                nc.sync.dma_start(out[kt * 128 : (kt + 1) * 128, :], dbg)

    nsplit = split_sync_waits(nc)
    print(f"split_sync_waits: {nsplit} NOPs inserted")
    return nc


def _bf16(a):
    return np.asarray(a, dtype=ml_dtypes.bfloat16)


def _fp8(a):
    return np.asarray(np.clip(a * WS, -240.0, 240.0), dtype=ml_dtypes.float8_e4m3)


def make_in_maps(x, tok_emb, pos_emb, wq, bq, wk, bk, wv, bv, wo, bo,
                 ln1_g, ln1_b, w1, b1, w2, b2, ln2_g, ln2_b, w_out, b_out):
    """Shard full inputs -> per-core input maps."""
    LE = wq.shape[0]
    per_r = []
    for r in range(TP):
        hs = slice(HL * r, HL * (r + 1))
        wqkv_r = np.concatenate(
            [
                wq[:, hs].transpose(0, 2, 1, 3).reshape(LE, D, HL * DH),
                wk[:, hs].transpose(0, 2, 1, 3).reshape(LE, D, HL * DH),
                wv[:, hs].transpose(0, 2, 1, 3).reshape(LE, D, HL * DH),
            ],
            axis=2,
        )
        bqkv_r = np.concatenate(
            [bq[:, hs].reshape(LE, -1), bk[:, hs].reshape(LE, -1),
             bv[:, hs].reshape(LE, -1)], axis=1,
        )  # [LE, 768]
        fs = slice(FFL * r, FFL * (r + 1))
        vs = slice(VL * r, VL * (r + 1))
        wout_r = np.zeros((D, VLP), np.float32)
        wout_r[:, :VL] = w_out[:, vs]
        bout_r = np.full((VLP,), -1e30, np.float32)
        bout_r[:VL] = b_out[vs]
        # packed per-layer bias/scale matrix [LE, NBM, 128]
        bmr = np.zeros((LE, NBM, 128), np.float32)
        bmr[:, BQKV:BQKV + 6, :] = bqkv_r.reshape(LE, 6, 128)
        bmr[:, BO:BO + KT, :] = (bo / TP).reshape(LE, KT, 128)
        bmr[:, G1:G1 + KT, :] = ln1_g.reshape(LE, KT, 128)
        bmr[:, BB1:BB1 + KT, :] = ln1_b.reshape(LE, KT, 128)
        bmr[:, B1C:B1C + KT, :] = b1[:, fs].reshape(LE, KT, 128)
        bmr[:, B2C:B2C + KT, :] = (b2 / TP).reshape(LE, KT, 128)
        bmr[:, G2:G2 + KT, :] = ln2_g.reshape(LE, KT, 128)
        bmr[:, BB2:BB2 + KT, :] = ln2_b.reshape(LE, KT, 128)
        bmr = np.ascontiguousarray(bmr.transpose(0, 2, 1))  # [LE, 128, NBM]
        per_r.append(dict(
            wqkv=_bf16(wqkv_r),
            wo=_bf16(wo[:, DH * HL * r : DH * HL * (r + 1), :]),
            w1=_bf16(w1[:, :, fs]),
            w2=_bf16(w2[:, fs, :]),
            bm=bmr,
            wout=_bf16(wout_r),
            bout=bout_r,
        ))
    in_maps = []
    for c in range(8):
        g, r = c // TP, c % TP
        emb = tok_emb[x[g]] + pos_emb[:S]          # [S, D]
        m = dict(per_r[r])
        m["h0"] = _bf16(np.ascontiguousarray(emb.T))
        in_maps.append(m)
    return in_maps


_CACHED = {}


def _ensure_ntff_hook():
    """The container's antenv package lacks axon_hooks; provide it so
    trace=True can drive NTFF profiling via the injected PJRT .so."""
    try:
        import antenv.axon_hooks  # noqa: F401
        return
    except ImportError:
        pass
    import sys
    import types

    mod = types.ModuleType("antenv.axon_hooks")
    state = {"fn": None}
    mod.set_axon_ntff_profile_hook = lambda fn: state.__setitem__("fn", fn)
    mod.get_axon_ntff_profile_hook = lambda: state["fn"]
    try:
        from trn_agent_boot.trn_boot import _ntff_profile_via_ctypes

        state["fn"] = _ntff_profile_via_ctypes("/opt/axon/libaxon_pjrt.so")
    except Exception:
        pass
    import antenv

    sys.modules["antenv.axon_hooks"] = mod
    antenv.axon_hooks = mod


def kernel(**inputs):
    inputs = {k: np.asarray(v) for k, v in inputs.items()}
    if "nc" not in _CACHED:
        _CACHED["nc"] = build_program()
    nc = _CACHED["nc"]
    in_maps = make_in_maps(**inputs)
    trace = os.environ.get("BASS_GPT_TRACE", "0") == "1"
    if trace:
        _ensure_ntff_hook()
    res = run_bass_kernel_spmd(
        nc, in_maps, core_ids=list(range(8)), trace=trace,
    )
    if trace:
        print(f"HW exec time: {res.exec_time_ns} ns")
        _CACHED["last_result"] = res
    results = res.results
    full = np.empty((B, S, V), np.float32)
    for c in range(8):
        g, r = c // TP, c % TP
        full[g, :, VL * r : VL * (r + 1)] = (
            results[c]["out"][:VL, :].astype(np.float32).T
        )
    return full


# revision 19
# speedup vs baseline: 1.0347x; 1.0347x over previous
"""GPT-style transformer forward on 8 Trainium2 NeuronCores.

Sharding: data-parallel over batch (2 groups of 4 cores), tensor-parallel
within each group (heads / FFN hidden / vocab columns split 4 ways).
Device activations are feature-major [feature, token] so all matmuls run
without transposes.

v2: everything is processed in 2 token blocks of 512 and software-pipelined
so the per-block AllReduces overlap the other block's compute.  Row
broadcasts run on GpSimd (partition_broadcast) instead of K=1 matmuls,
reciprocals use the fast-approx DVE op, causal masking is a GpSimd
affine_select directly on the exp() output, the residual accumulator is
bf16 so LN stats matmuls read it without a copy, and the vocab phase keeps
logits in SBUF with per-block log-softmax pipelining (bf16 output, host
casts to f32).
"""

import os
from contextlib import ExitStack

import numpy as np
import ml_dtypes

import concourse.bass as bass
import concourse.bass_utils as _bu

# walrus disables the LDWEIGHTS pull-ahead optimization by default; without
# it every matmul serializes behind its weight load (~+100ns/MM).  Re-enable
# it for this kernel's compiles (gated by env for A/B testing).
if os.environ.get("BASS_GPT_LDWOPT", "0") == "1":
    _orig_run_command = _bu.run_command

    def _run_command_ldw(cmd, **kw):
        cmd = [
            c.replace("--enable-ldw-opt=false", "--enable-ldw-opt=true")
            if isinstance(c, str) else c
            for c in cmd
        ]
        return _orig_run_command(cmd, **kw)

    _bu.run_command = _run_command_ldw
import concourse.mybir as mybir
import concourse.tile as tile
from concourse.bass_utils import run_bass_kernel_spmd
from concourse.vector_clock import ScopedClock


def _drain_and_barrier(self, tick_clock, wait_clock):
    """The walrus build here encodes Drain/NoOp as TPB_CTRL with at most one
    sync-wait slot; Tile's stock tail attaches all outstanding waits to the
    Drain and fails codegen. Split the waits one-per-NOP instead."""
    nop_inst = self.nc.sync.nop(nofuse=True)
    wait_clock.add_sem_waits(nop_inst.ins, ScopedClock({None: tick_clock.global_clock}))
    si = nop_inst.ins.sync_info
    if si is not None and len(si.on_wait) > 1:
        waits = list(si.on_wait)
        nop_inst.ins.sync_info = mybir.SyncInfo(on_wait=waits[:1], on_update=list(si.on_update))
        for w in waits[1:]:
            n2 = self.nc.sync.nop(nofuse=True)
            n2.ins.sync_info = mybir.SyncInfo(on_wait=[w], on_update=[])
    self.nc.sync.drain()
    self.nc.all_engine_barrier()
    assert self.sems is not None
    popped = self.nc._tile_sem_poison_stack.pop()
    assert popped is self._sem_poison
    self.nc.clear_and_free_semaphores(list(self.sems.allocated().values()))
    self.nc.all_engine_barrier()


tile.TileContext._drain_and_barrier = _drain_and_barrier

_MAX_WAITS = 1  # this walrus build caps sync-waits per instruction


def split_sync_waits(nc):
    """Hoist excess on_wait entries onto same-engine NOPs inserted before the
    instruction (engine queues execute in program order, so semantics hold)."""
    n = 0
    for bb in nc.main_func.blocks:
        insts = bb.instructions
        new_list = []
        for inst in insts:
            si = getattr(inst, "sync_info", None)
            # the LDW-opt codegen pass rejects Ldweights carrying sync waits;
            # hoist ALL of them (same-engine NOP preserves ordering).
            keep = 0 if isinstance(inst, mybir.InstLdweights) else _MAX_WAITS
            if si is not None and len(si.on_wait) > keep:
                waits = list(si.on_wait)
                cut = len(waits) - keep
                nops = []
                for w in waits[:cut]:
                    n += 1
                    nops.append(mybir.InstNoOp(
                        name=f"{inst.name}-sw{n}",
                        sync_info=mybir.SyncInfo(on_wait=[w], on_update=[]),
                        bass_nofuse=True,
                        engine=inst.engine,
                    ))
                # never split an Ldweights from its matmul: hoist the NOPs
                # above any trailing Ldweights run (waits only become more
                # conservative when moved earlier in an in-order queue)
                ip = len(new_list)
                while ip > 0 and isinstance(new_list[ip - 1], mybir.InstLdweights):
                    ip -= 1
                new_list[ip:ip] = nops
                if keep == 0 and not si.on_update:
                    inst.sync_info = None
                else:
                    inst.sync_info = mybir.SyncInfo(
                        on_wait=waits[cut:], on_update=list(si.on_update)
                    )
            new_list.append(inst)
        if len(new_list) != len(insts):
            bb.instructions[:] = new_list
    return n


# Model dims (hardcoded per problem spec)
L_FULL, H, D, V, SMAX = 8, 16, 1024, 32000, 1024
DH = D // H          # 64
FF = 4 * D           # 4096
B, S = 2, 1024
T = S                # tokens per group (one batch element per group)
TP = 4               # tensor-parallel degree within a group
HL = H // TP         # 4 local heads
FFL = FF // TP       # 1024 local FFN cols
VL = V // TP         # 8000 local vocab cols
VLP = 8064           # padded to 63*128
NVM = VLP // 128     # 63 vocab m-tiles
EPS = 1e-5
KT = D // 128        # 8 k-tiles over model dim
NB = T // 512        # 2 token blocks of 512

BF = mybir.dt.bfloat16
F8 = mybir.dt.float8e4
DRM = mybir.MatmulPerfMode.DoubleRow
KP = 4            # k-pair tiles (KT//2) for DoubleRow
WS = 1024.0       # fp8 weight scale
AS = 16.0         # fp8 activation scale
DS = 1.0 / (WS * AS)
F32 = mybir.dt.float32
AF = mybir.ActivationFunctionType
ALU = mybir.AluOpType

RG = [[0, 1, 2, 3], [4, 5, 6, 7]]

N_LAYERS = int(os.environ.get("BASS_GPT_LAYERS", str(L_FULL)))
SKIP_FINAL = os.environ.get("BASS_GPT_SKIP_FINAL", "0") == "1"

# packed per-layer bias/scale column offsets in bm_sb [128, 62]
BQKV, BO, G1, BB1, B1C, B2C, G2, BB2 = 0, 6, 14, 22, 30, 38, 46, 54
NBM = 62


def _r2(ap):
    """[ (kt p) n ] -> [p kt n] view of a DRAM 2-D tensor (p=128)."""
    return ap.rearrange("(kt p) n -> p kt n", p=128)


def build_program():
    nc = bass.Bass("TRN2")

    # ---- DRAM parameters (per-core shards) ----
    h0 = nc.declare_dram_parameter("h0", [D, T], BF, isOutput=False)
    wqkv = nc.declare_dram_parameter("wqkv", [N_LAYERS, D, 3 * HL * DH], BF, isOutput=False)
    wo = nc.declare_dram_parameter("wo", [N_LAYERS, HL * DH, D], BF, isOutput=False)
    w1 = nc.declare_dram_parameter("w1", [N_LAYERS, D, FFL], BF, isOutput=False)
    w2 = nc.declare_dram_parameter("w2", [N_LAYERS, FFL, D], BF, isOutput=False)
    bm = nc.declare_dram_parameter("bm", [N_LAYERS, 128, NBM], F32, isOutput=False)
    wout = nc.declare_dram_parameter("wout", [D, VLP], BF, isOutput=False)
    bout = nc.declare_dram_parameter("bout", [VLP], F32, isOutput=False)
    out = nc.declare_dram_parameter("out", [VLP, T], BF, isOutput=True)

    with ExitStack() as ctx:
        tc = ctx.enter_context(tile.TileContext(nc))

        const = ctx.enter_context(tc.tile_pool(name="const", bufs=1))
        hpool = ctx.enter_context(tc.tile_pool(name="hpool", bufs=1))
        rpool = ctx.enter_context(tc.tile_pool(name="rpool", bufs=4))
        dram = ctx.enter_context(tc.tile_pool(name="dram", bufs=2, space="DRAM"))

        # ---- constants ----
        ones_k = const.tile([128, 1], BF)       # lhsT for partition-sum (K=128, M=1)
        nc.vector.memset(ones_k, 1.0)
        eps_sb = const.tile([1, 1], F32)
        nc.vector.memset(eps_sb, float(D * D * EPS))
        ones_m = const.tile([1, 128], BF)       # lhsT for bf16 broadcast (K=1)
        nc.vector.memset(ones_m, 1.0)
        ones_mf = const.tile([1, 128], F32)     # lhsT for f32 broadcast
        nc.vector.memset(ones_mf, 1.0)
        lnD = const.tile([1, 1], F32)
        nc.vector.memset(lnD, float(np.log(D)))
        salt = os.environ.get("BASS_GPT_SALT", "")
        if salt:  # perturb the BIR so the NEFF cache misses on flag changes
            saltt = const.tile([1, 1], F32)
            nc.vector.memset(saltt, float(1 + len(salt)))

        # ---- persistent activation state ----
        hb = hpool.tile([128, KT, T], BF)       # residual stream (feature-major)
        nc.sync.dma_start(hb, _r2(h0))

        with ExitStack() as lctx:
            xpool = lctx.enter_context(tc.tile_pool(name="xpool", bufs=1))
            sqpool = lctx.enter_context(tc.tile_pool(name="sqpool", bufs=1))
            apool = lctx.enter_context(tc.tile_pool(name="apool", bufs=1))
            epool = lctx.enter_context(tc.tile_pool(name="epool", bufs=3))
            wq_pool = lctx.enter_context(tc.tile_pool(name="wq_pool", bufs=2))
            wf_pool = lctx.enter_context(tc.tile_pool(name="wf_pool", bufs=2))
            bpool = lctx.enter_context(tc.tile_pool(name="bpool", bufs=2))
            spool = lctx.enter_context(tc.tile_pool(name="spool", bufs=2))
            fpool = lctx.enter_context(tc.tile_pool(name="fpool", bufs=1))

            mm_psum = lctx.enter_context(tc.tile_pool(name="mm_psum", bufs=3, space="PSUM"))
            o_psum = lctx.enter_context(tc.tile_pool(name="o_psum", bufs=1, space="PSUM"))
            st_psum = lctx.enter_context(tc.tile_pool(name="st_psum", bufs=1, space="PSUM"))
            bc_psum = lctx.enter_context(tc.tile_pool(name="bc_psum", bufs=1, space="PSUM"))

            x1b = xpool.tile([128, KT, T], BF, tag="x1b")   # pre-LN accumulator
            qk_sb = apool.tile([128, 2, 2, T], BF)  # [part, q/k, head-pair, t]
            vaug = apool.tile([128, KT, HL, 65], BF)  # token-major V + ones col
            oT = apool.tile([128, 2, T], BF)        # attn head outputs (feature-major)
            f1 = fpool.tile([128, KT, T], BF)       # FFN hidden (local)

            def layernorm(blk, gcol, bcol, ar_out):
                """residual add + LN over features -> writes hb block (bf16).
                Per-kt pipelining: add (DVE) -> square (ACT) -> stats MMs so the
                stats start ~0.7us after the AllReduce readback instead of 5us."""
                tsl = slice(blk * 512, (blk + 1) * 512)
                ar_sb = spool.tile([128, KT, 512], BF, tag="stage")
                nc.sync.dma_start(ar_sb, _r2(ar_out))
                xs = sqpool.tile([128, KT, 512], BF, tag="xsq")
                s1p = st_psum.tile([1, 512], F32, tag="s1")
                s2p = st_psum.tile([33, 512], F32, tag="s2")
                for kt in range(KT):
                    nc.vector.tensor_add(x1b[:, kt, tsl], ar_sb[:, kt, :], hb[:, kt, tsl])
                    nc.scalar.activation(xs[:, kt, :], x1b[:, kt, tsl], AF.Square)
                    nc.tensor.matmul(s1p, ones_k, x1b[:, kt, tsl],
                                     start=(kt == 0), stop=(kt == KT - 1))
                    nc.tensor.matmul(s2p[32:33, :], ones_k, xs[:, kt, :],
                                     start=(kt == 0), stop=(kt == KT - 1))
                # u = s2*D - s1^2 + D^2*eps ; rstd = D/sqrt(u) = exp(-ln(u)/2 + ln D)
                t0 = rpool.tile([1, 512], F32, tag="row")
                nc.scalar.activation(t0, s1p, AF.Square)
                t1 = rpool.tile([1, 512], F32, tag="row")
                nc.vector.scalar_tensor_tensor(
                    out=t1, in0=s2p[32:33, :], scalar=float(D), in1=t0,
                    op0=ALU.mult, op1=ALU.subtract,
                )
                lu = rpool.tile([1, 512], F32, tag="row")
                nc.scalar.activation(lu, t1, AF.Ln, bias=eps_sb[0:1, 0:1])
                a2 = rpool.tile([1, 512], BF, tag="row")
                nc.scalar.activation(a2, lu, AF.Exp, scale=-0.5, bias=lnD[0:1, 0:1])
                c2 = rpool.tile([1, 512], BF, tag="row")
                nc.vector.scalar_tensor_tensor(
                    out=c2, in0=s1p, scalar=float(-1.0 / D), in1=a2,
                    op0=ALU.mult, op1=ALU.mult,
                )
                bcA = bc_psum.tile([128, 512], F32, tag="bc")
                nc.tensor.matmul(bcA, ones_m, a2, start=True, stop=True)
                A = spool.tile([128, 512], BF, tag="Abc")
                nc.vector.tensor_copy(A, bcA)
                bcC = bc_psum.tile([128, 512], F32, tag="bc")
                nc.tensor.matmul(bcC, ones_m, c2, start=True, stop=True)
                C = spool.tile([128, 512], BF, tag="Cbc")
                nc.vector.tensor_copy(C, bcC)
                for kt in range(KT):
                    Cp = spool.tile([128, 512], BF, tag="Cp")
                    nc.scalar.activation(
                        Cp, C, AF.Identity, scale=gcol[:, kt : kt + 1],
                        bias=bcol[:, kt : kt + 1],
                    )
                    u = spool.tile([128, 512], BF, tag="u")
                    nc.vector.scalar_tensor_tensor(
                        out=u, in0=x1b[:, kt, tsl], scalar=gcol[:, kt : kt + 1],
                        in1=A, op0=ALU.mult, op1=ALU.mult,
                    )
                    nc.vector.tensor_add(hb[:, kt, tsl], u, Cp)

            def load_attn_w(l):
                wqkv_sb = wq_pool.tile([128, KT, 768], BF, tag="wqkv")
                nc.sync.dma_start(wqkv_sb, _r2(wqkv[l]))
                wo_sb = wq_pool.tile([128, 2, D], BF, tag="wo")
                nc.sync.dma_start(wo_sb, _r2(wo[l]))
                bm_sb = bpool.tile([128, NBM], F32, tag="bm")
                nc.sync.dma_start(bm_sb, bm[l])
                return wqkv_sb, wo_sb, bm_sb

            def load_ffn_w(l):
                w1_sb = wf_pool.tile([128, KT, FFL], BF, tag="wf")
                nc.sync.dma_start(w1_sb, _r2(w1[l]))
                w2_sb = wf_pool.tile([128, KT, D], BF, tag="wf")
                nc.sync.dma_start(w2_sb, _r2(w2[l]))
                return w1_sb, w2_sb

            def qkv_attn_out(blk, W):
                """QKV + attention + out-proj + AllReduce for one token block."""
                wqkv_sb, wo_sb, bm_sb = W
                tsl = slice(blk * 512, (blk + 1) * 512)
                nc.gpsimd.memset(vaug[:, 4 * blk : 4 * blk + 4, :, 64:65], 1.0)
                for io in range(2):        # 0=q, 1=k (feature-major out)
                    for mt in range(2):    # head pair
                        mcol = (io * 2 + mt) * 128
                        ps = mm_psum.tile([128, 512], F32, tag="mm")
                        for kt in range(KT):
                            nc.tensor.matmul(
                                ps, wqkv_sb[:, kt, mcol : mcol + 128],
                                hb[:, kt, tsl],
                                start=(kt == 0), stop=(kt == KT - 1),
                            )
                        nc.scalar.activation(
                            qk_sb[:, io, mt, tsl], ps, AF.Identity,
                            bias=bm_sb[:, BQKV + io * 2 + mt : BQKV + io * 2 + mt + 1],
                        )
                for tm in range(4 * blk, 4 * blk + 4):  # v, token-major
                    ps = mm_psum.tile([128, 256], F32, tag="mm")
                    for kt in range(KT):
                        nc.tensor.matmul(
                            ps, hb[:, kt, tm * 128 : (tm + 1) * 128],
                            wqkv_sb[:, kt, 512:768],
                            start=(kt == 0), stop=(kt == KT - 1),
                        )
                    nc.scalar.activation(
                        vaug[:, tm, :, 0:64],
                        ps.rearrange("p (h e) -> p h e", h=HL), AF.Copy,
                    )
                # attention
                t1sl = tsl
                t2max = 4 * (blk + 1)
                for hm in range(2):
                    ps_oA = o_psum.tile([65, 512], F32, tag="oA")
                    ps_oB = o_psum.tile([65, 512], F32, tag="oB")
                    ps_o = [ps_oA, ps_oB]
                    ets = {}
                    for t2t in range(t2max + 1):
                        if t2t < t2max:
                            et = epool.tile([128, 2, 512], BF, tag="et")
                            ets[t2t] = et
                            for p in range(2):
                                pr = slice(64 * p, 64 * p + 64)
                                ps = mm_psum.tile([128, 512], F32, tag="mm")
                                nc.tensor.matmul(
                                    ps,
                                    qk_sb[pr, 1, hm, t2t * 128 : (t2t + 1) * 128],
                                    qk_sb[pr, 0, hm, t1sl],
                                    start=True, stop=True,
                                )
                                nc.scalar.activation(et[:, p, :], ps, AF.Exp, scale=0.125)
                            j = t2t - 4 * blk
                            if j >= 0:
                                nc.gpsimd.affine_select(
                                    out=et, in_=et,
                                    compare_op=ALU.is_ge, fill=0.0,
                                    base=-128 * j, pattern=[[0, 2], [1, 512]],
                                    channel_multiplier=-1,
                                )
                        if t2t >= 1:  # AV lags one tile so it never waits on exp
                            ta = t2t - 1
                            eta = ets.pop(ta)
                            for p in range(2):
                                nc.tensor.matmul(
                                    ps_o[p], vaug[:, ta, 2 * hm + p, :],
                                    eta[:, p, :],
                                    start=(ta == 0), stop=(ta == t2max - 1),
                                )
                    for p in range(2):
                        pr = slice(64 * p, 64 * p + 64)
                        lden = rpool.tile([1, 512], F32, tag="rec")
                        nc.scalar.activation(lden, ps_o[p][64:65, :], AF.Ln)
                        rcb = rpool.tile([1, 512], BF, tag="rec")
                        nc.scalar.activation(rcb, lden, AF.Exp, scale=-1.0)
                        bcR = bc_psum.tile([64, 512], F32, tag="bc")
                        nc.tensor.matmul(bcR, ones_m[:, 0:64], rcb, start=True, stop=True)
                        rb = rpool.tile([64, 512], BF, tag="rb")
                        nc.vector.tensor_copy(rb, bcR)
                        tmp = spool.tile([64, 512], BF, tag="otmp")
                        nc.vector.tensor_mul(tmp, ps_o[p][0:64, :], rb)
                        nc.vector.tensor_scalar(
                            out=oT[pr, hm, t1sl], in0=tmp,
                            scalar1=bm_sb[pr, BQKV + 4 + hm : BQKV + 5 + hm],
                            scalar2=None, op0=ALU.add,
                        )
                # out-proj + AllReduce
                stage = spool.tile([128, KT, 512], BF, tag="stage")
                for mt in range(KT):
                    ps = mm_psum.tile([128, 512], F32, tag="mm")
                    for kt in range(2):
                        nc.tensor.matmul(
                            ps, wo_sb[:, kt, mt * 128 : (mt + 1) * 128],
                            oT[:, kt, tsl],
                            start=(kt == 0), stop=(kt == 1),
                        )
                    nc.scalar.activation(
                        stage[:, mt, :], ps, AF.Identity,
                        bias=bm_sb[:, BO + mt : BO + mt + 1],
                    )
                ar_in = dram.tile([D, 512], BF, tag="arin")
                nc.sync.dma_start(_r2(ar_in), stage)
                ar_out = dram.tile([D, 512], BF, tag="arout")
                nc.gpsimd.collective_compute(
                    "AllReduce", ALU.add, replica_groups=RG,
                    ins=[ar_in.opt()], outs=[ar_out.opt()],
                )
                return ar_out

            def ln_ffn(blk, W, Wf, ar):
                """residual + LN1 + FFN + AllReduce for one token block."""
                bm_sb = W[2]
                w1_sb, w2_sb = Wf
                layernorm(blk, bm_sb[:, G1:G1 + KT], bm_sb[:, BB1:BB1 + KT], ar)
                tsl = slice(blk * 512, (blk + 1) * 512)
                for mt in range(KT):
                    ps = mm_psum.tile([128, 512], F32, tag="mm")
                    for kt in range(KT):
                        nc.tensor.matmul(
                            ps, w1_sb[:, kt, mt * 128 : (mt + 1) * 128],
                            hb[:, kt, tsl],
                            start=(kt == 0), stop=(kt == KT - 1),
                        )
                    nc.vector.tensor_scalar(
                        out=f1[:, mt, tsl], in0=ps,
                        scalar1=bm_sb[:, B1C + mt : B1C + mt + 1], scalar2=0.0,
                        op0=ALU.add, op1=ALU.max,
                    )
                stage = spool.tile([128, KT, 512], BF, tag="stage")
                for mt in range(KT):
                    ps = mm_psum.tile([128, 512], F32, tag="mm")
                    for kt in range(KT):
                        nc.tensor.matmul(
                            ps, w2_sb[:, kt, mt * 128 : (mt + 1) * 128],
                            f1[:, kt, tsl],
                            start=(kt == 0), stop=(kt == KT - 1),
                        )
                    nc.vector.tensor_scalar(
                        out=stage[:, mt, :], in0=ps,
                        scalar1=bm_sb[:, B2C + mt : B2C + mt + 1],
                        scalar2=None, op0=ALU.add,
                    )
                ar_in = dram.tile([D, 512], BF, tag="arin")
                nc.sync.dma_start(_r2(ar_in), stage)
                ar_out = dram.tile([D, 512], BF, tag="arout")
                nc.gpsimd.collective_compute(
                    "AllReduce", ALU.add, replica_groups=RG,
                    ins=[ar_in.opt()], outs=[ar_out.opt()],
                )
                return ar_out

            def ln2_block(blk, W, ar):
                bm_sb = W[2]
                layernorm(blk, bm_sb[:, G2:G2 + KT], bm_sb[:, BB2:BB2 + KT], ar)

            # software pipeline across blocks AND the layer boundary so the
            # Tensor stream always has AR-independent work queued behind
            # each collective (engine queues are static FIFOs).
            W = load_attn_w(0)
            arA0 = qkv_attn_out(0, W)
            for l in range(N_LAYERS):
                arA1 = qkv_attn_out(1, W)
                Wf = load_ffn_w(l)
                arF0 = ln_ffn(0, W, Wf, arA0)
                arF1 = ln_ffn(1, W, Wf, arA1)
                ln2_block(0, W, arF0)
                if l + 1 < N_LAYERS:
                    Wn = load_attn_w(l + 1)
                    arA0 = qkv_attn_out(0, Wn)
                    ln2_block(1, W, arF1)
                    W = Wn
                else:
                    ln2_block(1, W, arF1)

        # ---- Phase G: vocab projection + log-softmax (layer pools closed) ----
        if not SKIP_FINAL:
            with ExitStack() as gctx:
                gpool = gctx.enter_context(tc.tile_pool(name="gpool", bufs=1))
                wch_pool = gctx.enter_context(tc.tile_pool(name="wch", bufs=3))
                ebpool = gctx.enter_context(tc.tile_pool(name="eb", bufs=3))
                ospool = gctx.enter_context(tc.tile_pool(name="osp", bufs=4))
                gmm_psum = gctx.enter_context(tc.tile_pool(name="gmm", bufs=5, space="PSUM"))
                gst_psum = gctx.enter_context(tc.tile_pool(name="gst", bufs=1, space="PSUM"))
                gbc_psum = gctx.enter_context(tc.tile_pool(name="gbc", bufs=1, space="PSUM"))

                bout_sb = gpool.tile([128, NVM], F32)
                nc.sync.dma_start(bout_sb, bout.rearrange("(m p) -> p m", p=128))
                logits = gpool.tile([128, NVM, NB, 512], BF)
                for nb in range(NB):
                    tsl = slice(nb * 512, (nb + 1) * 512)
                    if nb == 0:
                        acc = gst_psum.tile([1, 512], F32, tag="acc0")
                        acc_ap = acc
                    else:
                        acc = gst_psum.tile([33, 512], F32, tag="acc1")
                        acc_ap = acc[32:33, :]
                    for vm in range(NVM):
                        wv_sb = wch_pool.tile([128, KT, 128], BF, tag="wch")
                        nc.sync.dma_start(wv_sb, _r2(wout)[:, :, vm * 128 : (vm + 1) * 128])
                        ps = gmm_psum.tile([128, 512], F32, tag="mm")
                        for kt in range(KT):
                            nc.tensor.matmul(
                                ps, wv_sb[:, kt, :], hb[:, kt, tsl],
                                start=(kt == 0), stop=(kt == KT - 1),
                            )
                        nc.vector.tensor_scalar(
                            out=logits[:, vm, nb, :], in0=ps,
                            scalar1=bout_sb[:, vm : vm + 1], scalar2=None, op0=ALU.add,
                        )
                        eb = ebpool.tile([128, 512], BF, tag="eb")
                        nc.scalar.activation(eb, ps, AF.Exp, bias=bout_sb[:, vm : vm + 1])
                        nc.tensor.matmul(
                            acc_ap, ones_k, eb,
                            start=(vm == 0), stop=(vm == NVM - 1), skip_group_check=True,
                        )
                    se_row = rpool.tile([1, 512], F32, tag="row")
                    nc.vector.tensor_copy(se_row, acc_ap)
                    se_in = dram.tile([1, 512], F32, tag="sein")
                    nc.sync.dma_start(se_in, se_row)
                    se_out = dram.tile([1, 512], F32, tag="seout")
                    nc.gpsimd.collective_compute(
                        "AllReduce", ALU.add, replica_groups=RG,
                        ins=[se_in.opt()], outs=[se_out.opt()],
                    )
                    se_sb = rpool.tile([1, 512], F32, tag="row")
                    nc.sync.dma_start(se_sb, se_out)
                    lr = rpool.tile([1, 512], BF, tag="row")
                    nc.scalar.activation(lr, se_sb, AF.Ln)
                    Lb = gbc_psum.tile([128, 512], F32, tag="gbc")
                    nc.tensor.matmul(Lb, ones_m, lr, start=True, stop=True)
                    Lsb = gpool.tile([128, 512], BF, tag=f"Lsb{nb}")
                    nc.vector.tensor_copy(Lsb, Lb)
                    for vm in range(NVM):
                        of = ospool.tile([128, 512], BF, tag="of")
                        nc.vector.tensor_sub(of, logits[:, vm, nb, :], Lsb)
                        nc.sync.dma_start(out[vm * 128 : (vm + 1) * 128, tsl], of)
        else:
            # debug: dump hb into out rows
            for kt in range(KT):
                dbg = spool.tile([128, T], BF, tag="stage")
                nc.scalar.activation(dbg, hb[:, kt, :], AF.Copy)
                nc.sync.dma_start(out[kt * 128 : (kt + 1) * 128, :], dbg)

    nsplit = split_sync_waits(nc)
    print(f"split_sync_waits: {nsplit} NOPs inserted")
    return nc


def _bf16(a):
    return np.asarray(a, dtype=ml_dtypes.bfloat16)


def _fp8(a):
    return np.asarray(np.clip(a * WS, -240.0, 240.0), dtype=ml_dtypes.float8_e4m3)


def make_in_maps(x, tok_emb, pos_emb, wq, bq, wk, bk, wv, bv, wo, bo,
                 ln1_g, ln1_b, w1, b1, w2, b2, ln2_g, ln2_b, w_out, b_out):
    """Shard full inputs -> per-core input maps."""
    LE = wq.shape[0]
    per_r = []
    for r in range(TP):
        hs = slice(HL * r, HL * (r + 1))
        wqkv_r = np.concatenate(
            [
                wq[:, hs].transpose(0, 2, 1, 3).reshape(LE, D, HL * DH),
                wk[:, hs].transpose(0, 2, 1, 3).reshape(LE, D, HL * DH),
                wv[:, hs].transpose(0, 2, 1, 3).reshape(LE, D, HL * DH),
            ],
            axis=2,
        )
        bqkv_r = np.concatenate(
            [bq[:, hs].reshape(LE, -1), bk[:, hs].reshape(LE, -1),
             bv[:, hs].reshape(LE, -1)], axis=1,
        )  # [LE, 768]
        fs = slice(FFL * r, FFL * (r + 1))
        vs = slice(VL * r, VL * (r + 1))
        wout_r = np.zeros((D, VLP), np.float32)
        wout_r[:, :VL] = w_out[:, vs]
        bout_r = np.full((VLP,), -1e30, np.float32)
        bout_r[:VL] = b_out[vs]
        # packed per-layer bias/scale matrix [LE, NBM, 128]
        bmr = np.zeros((LE, NBM, 128), np.float32)
        bmr[:, BQKV:BQKV + 6, :] = bqkv_r.reshape(LE, 6, 128)
        bmr[:, BO:BO + KT, :] = (bo / TP).reshape(LE, KT, 128)
        bmr[:, G1:G1 + KT, :] = ln1_g.reshape(LE, KT, 128)
        bmr[:, BB1:BB1 + KT, :] = ln1_b.reshape(LE, KT, 128)
        bmr[:, B1C:B1C + KT, :] = b1[:, fs].reshape(LE, KT, 128)
        bmr[:, B2C:B2C + KT, :] = (b2 / TP).reshape(LE, KT, 128)
        bmr[:, G2:G2 + KT, :] = ln2_g.reshape(LE, KT, 128)
        bmr[:, BB2:BB2 + KT, :] = ln2_b.reshape(LE, KT, 128)
        bmr = np.ascontiguousarray(bmr.transpose(0, 2, 1))  # [LE, 128, NBM]
        per_r.append(dict(
            wqkv=_bf16(wqkv_r),
            wo=_bf16(wo[:, DH * HL * r : DH * HL * (r + 1), :]),
            w1=_bf16(w1[:, :, fs]),
            w2=_bf16(w2[:, fs, :]),
            bm=bmr,
            wout=_bf16(wout_r),
            bout=bout_r,
        ))
    in_maps = []
    for c in range(8):
        g, r = c // TP, c % TP
        emb = tok_emb[x[g]] + pos_emb[:S]          # [S, D]
        m = dict(per_r[r])
        m["h0"] = _bf16(np.ascontiguousarray(emb.T))
        in_maps.append(m)
    return in_maps


_CACHED = {}


def _ensure_ntff_hook():
    """The container's antenv package lacks axon_hooks; provide it so
    trace=True can drive NTFF profiling via the injected PJRT .so."""
    try:
        import antenv.axon_hooks  # noqa: F401
        return
    except ImportError:
        pass
    import sys
    import types

    mod = types.ModuleType("antenv.axon_hooks")
    state = {"fn": None}
    mod.set_axon_ntff_profile_hook = lambda fn: state.__setitem__("fn", fn)
    mod.get_axon_ntff_profile_hook = lambda: state["fn"]
    try:
        from trn_agent_boot.trn_boot import _ntff_profile_via_ctypes

        state["fn"] = _ntff_profile_via_ctypes("/opt/axon/libaxon_pjrt.so")
    except Exception:
        pass
    import antenv

    sys.modules["antenv.axon_hooks"] = mod
    antenv.axon_hooks = mod


def kernel(**inputs):
    inputs = {k: np.asarray(v) for k, v in inputs.items()}
    if "nc" not in _CACHED:
        _CACHED["nc"] = build_program()
    nc = _CACHED["nc"]
    in_maps = make_in_maps(**inputs)
    trace = os.environ.get("BASS_GPT_TRACE", "0") == "1"
    if trace:
        _ensure_ntff_hook()
    res = run_bass_kernel_spmd(
        nc, in_maps, core_ids=list(range(8)), trace=trace,
    )
    if trace:
        print(f"HW exec time: {res.exec_time_ns} ns")
        _CACHED["last_result"] = res
    results = res.results
    full = np.empty((B, S, V), np.float32)
    for c in range(8):
        g, r = c // TP, c % TP
        full[g, :, VL * r : VL * (r + 1)] = (
            results[c]["out"][:VL, :].astype(np.float32).T
        )
    return full
